# revision 24
# baseline (speedup 1.0000x reference)
"""Multi-head attention (B=4, S=2048, D=2048, H=16) on 8 trn2 NeuronCores.

Sharding: tensor-parallel over heads - 2 heads per core. Each core computes
its heads' Q/K/V projections, full attention for those heads, and a partial
output projection (its 256 rows of wo). The host sums the 8 fp16 partial
outputs (plus the bias terms: bv folds through softmax-rows-sum-to-1, bo is
added once).

Design (947us baseline -> 767us):
  - fp16 datapath: every matmul operand (x, weights, Q/K/V, exp tiles,
    attention output) is fp16; PSUM accumulation stays fp32. Same relative
    precision as tf32 (10 mantissa bits) at half the DMA/SBUF/LDWEIGHTS
    cost. rel_err ~1.2e-3 vs the 2e-2 gate.
  - softmax denominator off the PE: exp tiles are pair-summed on the
    otherwise idle GPSIMD (T0=p0+p1, T1=p2+p3; latency-tolerant), combined
    on the DVE (U=T0+T1, T2=p4+p5, X=U+T2, then T3=p6+p7, X2=X+T3 and a
    half-fold X3 in the tail), so a SINGLE ones-matmul per half-span — run
    in the next span's kp0 slot, where the avt consumer is still ~10us
    away — partition-reduces the whole denominator. This replaces the
    original 512 denominator matmuls (~119us of PE; the previous revision
    still used 6 per half-span, ~41us).
  - the phase-B pair loop is paced by the ACT exp stream (1113ns per
    [128,1024] exp + ~180ns dispatch restart). The PE is kept saturated
    against that cadence: the out-projection of span qs-1 is emitted as 16
    two-matmul groups pulled into span qs's pair loops, and each upcoming
    half-span's first two scores+exp pairs are emitted before the current
    tail ("warm pairs") so the exp stream never restarts cold across
    spans, heads, or batches.
  - PSUM (8 banks): scores ring 2x[128,1024] (4), av ring 2 (2, its own
    tag so a new half-span's AV never waits the previous normalize),
    proj/outproj/dn shared ring 2 (2).
  - PSUM->SBUF output copies ride the DVE; the final drain splits each
    copy across DVE+ACT halves (both idle at the end).
  - b=0 startup: wq chunk-DMAs interleave with the first x span; Q/K run
    as alternating half-passes so the PE never outruns the 320GB/s x
    stream; first matmul at ~12.5us.
"""
import os
import sys

sys.path.insert(0, "/opt/trn_rl_repo")
import numpy as np

B, S, D, H = 4, 2048, 2048, 16
HD = 128
NCORES = 8
HP = H // NCORES          # heads per core = 2
DC = HP * HD              # per-core slice of D = 256
TOK = B * S               # 8192
SCALE = HD ** -0.5
NDC = D // 128            # 16 contraction chunks for the projections
SPAN = 256                # token span per projection step
NSPAN = S // SPAN         # 8 spans per batch
QS = 512                  # query span in attention
NQS = S // QS             # 4
NKC = S // 128            # 16 key chunks
NPAIR = NKC // 2          # 8 key-chunk pairs

LAST_EXEC_NS = None
_BUILT = None


def _build():
    global _BUILT
    if _BUILT is not None:
        return _BUILT
    import concourse.tile as tile
    from concourse import bacc, mybir

    F16 = mybir.dt.float16
    F32 = mybir.dt.float32
    Exp = mybir.ActivationFunctionType.Exp
    Ident = mybir.ActivationFunctionType.Identity

    nc = bacc.Bacc("TRN2", target_bir_lowering=False, debug=False)
    # All inputs are pre-swizzled on the host so every DMA reads contiguous
    # multi-KB per-partition rows (512B strided pieces run at ~40% of peak).
    # xt: per (batch,span) block of [128 part, NDC*SPAN] contiguous.
    xt = nc.dram_tensor("xt", [B * NSPAN * 128, NDC * SPAN], F16,
                        kind="ExternalInput")
    wq = nc.dram_tensor("wq", [128, NDC * DC], F16, kind="ExternalInput")
    wk = nc.dram_tensor("wk", [128, NDC * DC], F16, kind="ExternalInput")
    wv = nc.dram_tensor("wv", [128, NDC * DC], F16, kind="ExternalInput")
    wo = nc.dram_tensor("wo", [128, HP * D], F16, kind="ExternalInput")
    bqk = nc.dram_tensor("bqk", [HD, 2 * HP], F32, kind="ExternalInput")
    out = nc.dram_tensor("out", [TOK, D], F16, kind="ExternalOutput")

    with tile.TileContext(nc) as tc:
        with tc.tile_pool(name="const", bufs=1) as cpool, \
             tc.tile_pool(name="xp", bufs=8) as xpool, \
             tc.tile_pool(name="bt", bufs=2) as bpool, \
             tc.tile_pool(name="qk", bufs=2) as qkpool, \
             tc.tile_pool(name="avp", bufs=2) as avpool, \
             tc.tile_pool(name="pp", bufs=5) as ppool, \
             tc.tile_pool(name="s2", bufs=8) as s2pool, \
             tc.tile_pool(name="rc", bufs=2) as rpool, \
             tc.tile_pool(name="ot", bufs=2) as opool, \
             tc.tile_pool(name="ps", bufs=1, space="PSUM") as ps:

            wq_sb = cpool.tile([128, NDC, DC], F16)
            wk_sb = cpool.tile([128, NDC, DC], F16)
            wv_sb = cpool.tile([128, NDC, DC], F16)
            wo_sb = cpool.tile([128, HP, D], F16)
            ones_sb = cpool.tile([128, 128], F16)
            bqk_sb = cpool.tile([HD, 2 * HP], F32)

            xts = {}

            def x_dma(b, sp, eng=None):
                r0 = (b * NSPAN + sp) * 128
                xtl = xpool.tile([128, NDC, SPAN], F16, name=f"x{b}_{sp}",
                                 tag="x")
                xts[(b, sp)] = xtl
                (eng or nc.sync).dma_start(out=xtl, in_=xt[r0:r0 + 128, :])

            # --- b=0 startup. sync queue: biases first (tiny descriptors,
            # needed by the first activation ~12us in), then wq, wk. scalar
            # queue: batch-0 x spans, then wv/wo (needed only at ~60/~100us).
            # First ~20us of HBM traffic stays under ~6MB so the Q pass is
            # never DMA-starved. ---
            x00 = xpool.tile([128, NDC, SPAN], F16, name="x0_0", tag="x")
            xts[(0, 0)] = x00
            nc.sync.dma_start(out=bqk_sb, in_=bqk[:, :])
            for i in range(4):
                nc.sync.dma_start(out=wq_sb[:, 4 * i:4 * i + 4, :],
                                  in_=wq[:, 1024 * i:1024 * (i + 1)])
                nc.scalar.dma_start(out=x00[:, 4 * i:4 * i + 4, :],
                                    in_=xt[0:128, 1024 * i:1024 * (i + 1)])
            nc.vector.memset(ones_sb, 1.0)
            for i in range(2):
                nc.sync.dma_start(out=wk_sb[:, 8 * i:8 * i + 8, :],
                                  in_=wk[:, 2048 * i:2048 * (i + 1)])
            for sp in range(1, NSPAN):
                x_dma(0, sp, eng=nc.scalar)
            for i in range(2):
                nc.scalar.dma_start(out=wv_sb[:, 8 * i:8 * i + 8, :],
                                    in_=wv[:, 2048 * i:2048 * (i + 1)])
            nc.scalar.dma_start(out=wo_sb, in_=wo[:, :])

            def pull(filler):
                if filler is not None:
                    next(filler, None)

            def proj_pass(b, w_sb, b_off, dst, spans=None):
                for sp in (spans if spans is not None else range(NSPAN)):
                    xtl = xts[(b, sp)]
                    for h in range(HP):
                        pps = ps.tile([128, SPAN], F32, name="pps", tag="pj",
                                      bufs=2)
                        for c in range(NDC):
                            nc.tensor.matmul(
                                pps, w_sb[:, c, h * HD:(h + 1) * HD],
                                xtl[:, c, :], start=(c == 0),
                                stop=(c == NDC - 1))
                        nc.scalar.activation(
                            dst[:, h, sp * SPAN:(sp + 1) * SPAN], pps, Ident,
                            bias=bqk_sb[:, b_off + h:b_off + h + 1])

            def v_pass(b, v_b):
                for sp in range(NSPAN):
                    xtl = xts[(b, sp)]
                    vps = ps.tile([128, 2 * DC], F32, name="vps", tag="pj",
                                  bufs=2)
                    for tch in range(2):
                        for c in range(NDC):
                            nc.tensor.matmul(
                                vps[:, tch * DC:(tch + 1) * DC],
                                xtl[:, c, tch * 128:(tch + 1) * 128],
                                wv_sb[:, c, :], start=(c == 0),
                                stop=(c == NDC - 1))
                    for tch in range(2):
                        nc.scalar.copy(v_b[:, sp * 2 + tch, :],
                                       vps[:, tch * DC:(tch + 1) * DC])

            def warm_pair(qs, h, kp, qt_b, kt_b):
                # scores+exp of an upcoming half-span pair, emitted before
                # the current tail so the ACT exp stream never restarts cold
                q_sl = qt_b[:, h, qs * QS:(qs + 1) * QS]
                s_ps = ps.tile([128, 2 * QS], F32, name="s_ps", tag="s",
                               bufs=2)
                for j in range(2):
                    kc = 2 * kp + j
                    nc.tensor.matmul(
                        s_ps[:, j * QS:(j + 1) * QS],
                        kt_b[:, h, kc * 128:(kc + 1) * 128], q_sl,
                        start=True, stop=True)
                pt = ppool.tile([128, 2 * QS], F16, name="p_sb", tag="p")
                nc.scalar.activation(pt, s_ps, Exp, scale=SCALE)
                return pt

            def attn_span(qs, h, qt_b, kt_b, v_b, avt_b, filler=None,
                          warm=None, fin=None):
                q_sl = qt_b[:, h, qs * QS:(qs + 1) * QS]
                av_ps = ps.tile([HD, QS], F32, name="av_ps", tag="av",
                                bufs=2)
                p_tiles = []

                def emit_av(kp):
                    pt = p_tiles[kp]
                    for j in range(2):
                        kc = 2 * kp + j
                        nc.tensor.matmul(
                            av_ps, v_b[:, kc, h * HD:(h + 1) * HD],
                            pt[:, j * QS:(j + 1) * QS], start=(kc == 0),
                            stop=(kc == NKC - 1))

                tre = {}

                def s2t(nm, wide=True):
                    return s2pool.tile([128, (2 * QS) if wide else QS], F16,
                                       name=nm, tag="s2")

                for kp in range(NPAIR):
                    if kp == 0 and fin is not None:
                        fin()      # previous span's AV tail + dn + normalize
                    if warm is not None and kp < len(warm):
                        p_tiles.append(warm[kp])
                    else:
                        s_ps = ps.tile([128, 2 * QS], F32, name="s_ps",
                                       tag="s", bufs=2)
                        for j in range(2):
                            kc = 2 * kp + j
                            nc.tensor.matmul(
                                s_ps[:, j * QS:(j + 1) * QS],
                                kt_b[:, h, kc * 128:(kc + 1) * 128], q_sl,
                                start=True, stop=True)
                        pt = ppool.tile([128, 2 * QS], F16, name="p_sb",
                                        tag="p")
                        nc.scalar.activation(pt, s_ps, Exp, scale=SCALE)
                        p_tiles.append(pt)
                    # pair-sum tree: three pair-adds on the idle GPSIMD
                    # (latency-tolerant), combines + half-folds on the DVE,
                    # scheduled so x3 (the folded full denominator row-set)
                    # is ready ~1.1us after the last exp; the single
                    # ones-matmul runs in the next span's kp0 slot
                    if kp in (1, 3, 5):
                        tre[kp] = s2t("t_sb")
                        nc.gpsimd.tensor_add(tre[kp], p_tiles[kp - 1],
                                             p_tiles[kp])
                    elif kp == 4:
                        tre["u"] = s2t("u_sb")
                        nc.vector.tensor_add(tre["u"], tre[1], tre[3])
                    elif kp == 7:
                        tre["x"] = s2t("x_sb")
                        nc.vector.tensor_add(tre["x"], tre["u"], tre[5])
                        tre["xf"] = s2t("xf_sb", wide=False)
                        nc.vector.tensor_add(tre["xf"], tre["x"][:, 0:QS],
                                             tre["x"][:, QS:2 * QS])
                        tre["a"] = s2t("a_sb", wide=False)
                        nc.vector.tensor_add(tre["a"], p_tiles[6][:, 0:QS],
                                             p_tiles[6][:, QS:2 * QS])
                    if kp >= 2:
                        pull(filler)
                    if kp == 3:
                        emit_av(0)
                        emit_av(1)
                    elif kp >= 4:
                        emit_av(kp - 2)

                def tail(mid=None):
                    bf = s2t("bf_sb", wide=False)
                    nc.vector.tensor_add(bf, p_tiles[7][:, 0:QS],
                                         p_tiles[7][:, QS:2 * QS])
                    ab = s2t("ab_sb", wide=False)
                    nc.vector.tensor_add(ab, tre["a"], bf)
                    x3 = s2t("x3_sb", wide=False)
                    nc.vector.tensor_add(x3, tre["xf"], ab)
                    if mid is not None:
                        mid()
                    pull(filler)
                    pull(filler)

                    def fin():
                        # deferred span finish, run in the NEXT span's kp0-1
                        # slot where the PE would otherwise starve waiting on
                        # the exp stream: last two AV pairs, then the single
                        # denominator ones-matmul + normalize
                        emit_av(NPAIR - 2)
                        emit_av(NPAIR - 1)
                        dn_ps = ps.tile([128, QS], F32, name="dn_ps",
                                        tag="pj", bufs=2)
                        nc.tensor.matmul(dn_ps, ones_sb, x3, start=True,
                                         stop=True)
                        recip = rpool.tile([128, QS], F32, name="recip",
                                           tag="rc")
                        nc.vector.reciprocal_approx_fast(recip, dn_ps)
                        nc.vector.tensor_mul(
                            avt_b[:, h, qs * QS:(qs + 1) * QS], av_ps, recip)
                    return fin
                return tail

            def outproj_gen(b, qs, avt_b, split, final=False):
                for tloc in range(QS // 128):
                    tch = qs * (QS // 128) + tloc
                    out_sb = opool.tile([128, D], F16, name="out_sb",
                                        tag="ot")
                    rows = slice(b * S + tch * 128, b * S + (tch + 1) * 128)
                    for dsp in range(4):
                        ops = ps.tile([128, 512], F32, name="ops", tag="pj",
                                      bufs=2)
                        for h in range(HP):
                            nc.tensor.matmul(
                                ops, avt_b[:, h, tch * 128:(tch + 1) * 128],
                                wo_sb[:, h, dsp * 512:(dsp + 1) * 512],
                                start=(h == 0), stop=(h == HP - 1))
                        if split[dsp] == "v":
                            nc.vector.tensor_copy(
                                out_sb[:, dsp * 512:(dsp + 1) * 512], ops)
                        elif split[dsp] == "s":
                            nc.scalar.copy(
                                out_sb[:, dsp * 512:(dsp + 1) * 512], ops)
                        else:   # "2": halves on both engines in parallel
                            nc.vector.tensor_copy(
                                out_sb[:, dsp * 512:dsp * 512 + 256],
                                ops[:, 0:256])
                            nc.scalar.copy(
                                out_sb[:, dsp * 512 + 256:(dsp + 1) * 512],
                                ops[:, 256:512])
                        if final:
                            # drain: ship each quarter as soon as it's ready
                            # so the last DMA starts right after the last copy
                            nc.sync.dma_start(
                                out=out[rows, dsp * 512:(dsp + 1) * 512],
                                in_=out_sb[:, dsp * 512:(dsp + 1) * 512])
                        elif dsp == 3:
                            nc.sync.dma_start(out=out[rows, :], in_=out_sb)
                        yield

            carry = None          # half-consumed outproj of (b-1, qs=3)
            prev_fin = None       # previous span's dn+normalize closure
            for b in range(B):
                qt_b = qkpool.tile([128, HP, S], F16, name="qt_b", tag="qt")
                kt_b = qkpool.tile([128, HP, S], F16, name="kt_b", tag="kt")
                v_b = bpool.tile([128, NKC, DC], F16, name="v_b", tag="v")
                avt_b = avpool.tile([128, HP, S], F16, name="avt_b",
                                    tag="avt")

                if b == 0:
                    # first batch is DMA-paced: alternate Q/K half-passes so
                    # the PE never outruns the x-span stream
                    proj_pass(b, wq_sb, 0, qt_b, spans=range(0, 4))
                    proj_pass(b, wk_sb, HP, kt_b, spans=range(0, 4))
                    proj_pass(b, wq_sb, 0, qt_b, spans=range(4, 8))
                    proj_pass(b, wk_sb, HP, kt_b, spans=range(4, 8))
                else:
                    proj_pass(b, wq_sb, 0, qt_b)
                    proj_pass(b, wk_sb, HP, kt_b)
                v_pass(b, v_b)

                warm = [warm_pair(0, 0, 0, qt_b, kt_b),
                        warm_pair(0, 0, 1, qt_b, kt_b)]
                for qs in range(NQS):
                    if qs == 0:
                        filler = carry       # leftovers (may be exhausted)
                    else:
                        filler = outproj_gen(b, qs - 1, avt_b, "vvvv")
                    tail = attn_span(qs, 0, qt_b, kt_b, v_b, avt_b, filler,
                                     warm, fin=prev_fin)
                    warm = [warm_pair(qs, 1, 0, qt_b, kt_b)]
                    prev_fin = tail(mid=lambda: warm.append(
                        warm_pair(qs, 1, 1, qt_b, kt_b)))
                    if qs == 0 and b + 1 < B:
                        for sp in range(NSPAN):
                            x_dma(b + 1, sp)
                    tail = attn_span(qs, 1, qt_b, kt_b, v_b, avt_b, filler,
                                     warm, fin=prev_fin)
                    if qs + 1 < NQS:
                        warm = [warm_pair(qs + 1, 0, 0, qt_b, kt_b)]
                        prev_fin = tail(mid=lambda q=qs: warm.append(
                            warm_pair(q + 1, 0, 1, qt_b, kt_b)))
                    else:
                        warm = None
                        prev_fin = tail()
                    if filler is not None:
                        for _ in filler:     # drain any leftovers
                            pass
                carry = outproj_gen(b, NQS - 1, avt_b,
                                    "vvvv" if b + 1 < B else "svsv",
                                    final=(b + 1 == B))

            if carry is not None:            # last batch's final span:
                prev_fin()                       # normalize its avt first,
                for _ in carry:                  # then drain with copies
                    pass                         # split across ACT+DVE
    nc.compile()
    _BUILT = nc
    return nc


def _install_trace_hooks():
    import types
    try:
        import antenv.axon_hooks  # noqa: F401
        return True
    except ImportError:
        pass
    try:
        from trn_agent_boot.trn_boot import _ntff_profile_via_ctypes
        hook = _ntff_profile_via_ctypes('/opt/axon/libaxon_pjrt.so')
        if hook is None:
            return False
        m = types.ModuleType('antenv.axon_hooks')
        m.get_axon_ntff_profile_hook = lambda: hook
        sys.modules['antenv.axon_hooks'] = m
        from concourse import bass_utils
        bass_utils.upload_artifacts = lambda tmpdir: "local://" + tmpdir
        return True
    except Exception:
        return False


def kernel(x, wq, bq, wk, bk, wv, bv, wo, bo):
    global LAST_EXEC_NS
    from concourse.bass_utils import run_bass_kernel_spmd

    x = np.asarray(x, dtype=np.float32)
    wq = np.asarray(wq, dtype=np.float32)
    bq = np.asarray(bq, dtype=np.float32)
    wk = np.asarray(wk, dtype=np.float32)
    bk = np.asarray(bk, dtype=np.float32)
    wv = np.asarray(wv, dtype=np.float32)
    bv = np.asarray(bv, dtype=np.float32)
    wo = np.asarray(wo, dtype=np.float32)
    bo = np.asarray(bo, dtype=np.float32)

    # xt blocks: [(b,sp), 128 part, NDC, SPAN] with contiguous 8KB
    # per-partition rows; element (b, sp, p, c, t) = x[b, sp*SPAN+t, c*128+p]
    xt = np.ascontiguousarray(
        x.reshape(B, NSPAN, SPAN, NDC, 128).transpose(0, 1, 4, 3, 2)
    ).astype(np.float16).reshape(B * NSPAN * 128, NDC * SPAN)

    def w_prep(w):  # [D, DC] -> [128, NDC*DC] (partition-major, contiguous)
        return np.ascontiguousarray(
            w.reshape(NDC, 128, DC).transpose(1, 0, 2)
        ).astype(np.float16).reshape(128, NDC * DC)

    in_maps = []
    for i in range(NCORES):
        sl = slice(i * DC, (i + 1) * DC)
        in_maps.append({
            "xt": xt,
            "wq": w_prep(wq[:, sl]),
            "wk": w_prep(wk[:, sl]),
            "wv": w_prep(wv[:, sl]),
            "wo": np.ascontiguousarray(
                wo[sl, :].reshape(HP, 128, D).transpose(1, 0, 2)
            ).astype(np.float16).reshape(128, HP * D),
            "bqk": np.ascontiguousarray(np.concatenate(
                [bq[sl].reshape(HP, HD).T, bk[sl].reshape(HP, HD).T],
                axis=1)),
        })

    trace = bool(os.environ.get("KERNEL_TRACE"))
    if trace:
        trace = _install_trace_hooks()

    nc = _build()
    res = run_bass_kernel_spmd(nc, in_maps, list(range(NCORES)), trace=trace)
    LAST_EXEC_NS = res.exec_time_ns

    total = np.zeros((TOK, D), dtype=np.float32)
    for r in res.results:
        total += r["out"]
    # V-bias folds into a constant row: softmax rows sum to 1, so
    # attention(V + 1*bv^T) = attention(V) + 1*bv^T, and (bv @ wo) adds to bo.
    total += bo + bv @ wo
    return total.reshape(B, S, D)



# revision 26
# speedup vs baseline: 1.0762x; 1.0762x over previous
"""Multi-head attention (B=4, S=2048, D=2048, H=16) on 8 trn2 NeuronCores.

Sharding: tensor-parallel over heads - 2 heads per core. Each core computes
its heads' Q/K/V projections, full attention for those heads, and a partial
output projection (its 256 rows of wo). The host sums the 8 fp16 partial
outputs (plus the bias terms: bv folds through softmax-rows-sum-to-1, bo is
added once).

Design (947us baseline -> 767us):
  - fp16 datapath: every matmul operand (x, weights, Q/K/V, exp tiles,
    attention output) is fp16; PSUM accumulation stays fp32. Same relative
    precision as tf32 (10 mantissa bits) at half the DMA/SBUF/LDWEIGHTS
    cost. rel_err ~1.2e-3 vs the 2e-2 gate.
  - softmax denominator off the PE: exp tiles are pair-summed on the
    otherwise idle GPSIMD (T0=p0+p1, T1=p2+p3; latency-tolerant), combined
    on the DVE (U=T0+T1, T2=p4+p5, X=U+T2, then T3=p6+p7, X2=X+T3 and a
    half-fold X3 in the tail), so a SINGLE ones-matmul per half-span — run
    in the next span's kp0 slot, where the avt consumer is still ~10us
    away — partition-reduces the whole denominator. This replaces the
    original 512 denominator matmuls (~119us of PE; the previous revision
    still used 6 per half-span, ~41us).
  - the phase-B pair loop is paced by the ACT exp stream (1113ns per
    [128,1024] exp + ~180ns dispatch restart). The PE is kept saturated
    against that cadence: the out-projection of span qs-1 is emitted as 16
    two-matmul groups pulled into span qs's pair loops, and each upcoming
    half-span's first two scores+exp pairs are emitted before the current
    tail ("warm pairs") so the exp stream never restarts cold across
    spans, heads, or batches.
  - PSUM (8 banks): scores ring 2x[128,1024] (4), av ring 2 (2, its own
    tag so a new half-span's AV never waits the previous normalize),
    proj/outproj/dn shared ring 2 (2).
  - PSUM->SBUF output copies ride the DVE; the final drain splits each
    copy across DVE+ACT halves (both idle at the end).
  - b=0 startup: wq chunk-DMAs interleave with the first x span; Q/K run
    as alternating half-passes so the PE never outruns the 320GB/s x
    stream; first matmul at ~12.5us.
"""
import os
import sys

sys.path.insert(0, "/opt/trn_rl_repo")
import numpy as np

B, S, D, H = 4, 2048, 2048, 16
HD = 128
NCORES = 8
HP = H // NCORES          # heads per core = 2
DC = HP * HD              # per-core slice of D = 256
TOK = B * S               # 8192
SCALE = HD ** -0.5
NDC = D // 128            # 16 contraction chunks for the projections
SPAN = 256                # token span per projection step
NSPAN = S // SPAN         # 8 spans per batch
QS = 512                  # query span in attention
NQS = S // QS             # 4
NKC = S // 128            # 16 key chunks
NPAIR = NKC // 2          # 8 key-chunk pairs

LAST_EXEC_NS = None
_BUILT = None


def _build():
    global _BUILT
    if _BUILT is not None:
        return _BUILT
    import concourse.tile as tile
    from concourse import bacc, mybir

    F16 = mybir.dt.float16
    F32 = mybir.dt.float32
    Exp = mybir.ActivationFunctionType.Exp
    Ident = mybir.ActivationFunctionType.Identity

    nc = bacc.Bacc("TRN2", target_bir_lowering=False, debug=False)
    # All inputs are pre-swizzled on the host so every DMA reads contiguous
    # multi-KB per-partition rows (512B strided pieces run at ~40% of peak).
    # xt: per (batch,span) block of [128 part, NDC*SPAN] contiguous.
    xt = nc.dram_tensor("xt", [B * NSPAN * 128, NDC * SPAN], F16,
                        kind="ExternalInput")
    wq = nc.dram_tensor("wq", [128, NDC * DC], F16, kind="ExternalInput")
    wk = nc.dram_tensor("wk", [128, NDC * DC], F16, kind="ExternalInput")
    wv = nc.dram_tensor("wv", [128, NDC * DC], F16, kind="ExternalInput")
    wo = nc.dram_tensor("wo", [128, HP * D], F16, kind="ExternalInput")
    bqk = nc.dram_tensor("bqk", [HD, 2 * HP], F32, kind="ExternalInput")
    out = nc.dram_tensor("out", [TOK, D], F16, kind="ExternalOutput")

    with tile.TileContext(nc) as tc:
        with tc.tile_pool(name="const", bufs=1) as cpool, \
             tc.tile_pool(name="xp", bufs=8) as xpool, \
             tc.tile_pool(name="bt", bufs=2) as bpool, \
             tc.tile_pool(name="qk", bufs=2) as qkpool, \
             tc.tile_pool(name="avp", bufs=2) as avpool, \
             tc.tile_pool(name="pp", bufs=5) as ppool, \
             tc.tile_pool(name="s2", bufs=8) as s2pool, \
             tc.tile_pool(name="rc", bufs=2) as rpool, \
             tc.tile_pool(name="ot", bufs=2) as opool, \
             tc.tile_pool(name="ps", bufs=1, space="PSUM") as ps:

            wq_sb = cpool.tile([128, NDC, DC], F16)
            wk_sb = cpool.tile([128, NDC, DC], F16)
            wv_sb = cpool.tile([128, NDC, DC], F16)
            wo_sb = cpool.tile([128, HP, D], F16)
            ones_sb = cpool.tile([128, 128], F16)
            bqk_sb = cpool.tile([HD, 2 * HP], F32)

            xts = {}

            def x_dma(b, sp, eng=None):
                r0 = (b * NSPAN + sp) * 128
                xtl = xpool.tile([128, NDC, SPAN], F16, name=f"x{b}_{sp}",
                                 tag="x")
                xts[(b, sp)] = xtl
                (eng or nc.sync).dma_start(out=xtl, in_=xt[r0:r0 + 128, :])

            # --- b=0 startup. A HWDGE dma_start occupies the issuing
            # engine's queue and can BLOCK it waiting for a free DMA sem
            # lane, so the scalar (ACT) queue gets only the first x spans —
            # ACT must be free for the first proj activations by ~12us.
            # Everything else rides sync in need-order: bias (first act),
            # wq, wk (K pass ~25us), x2-x7 (~22-45us), wv (~65), wo (~100).
            x00 = xpool.tile([128, NDC, SPAN], F16, name="x0_0", tag="x")
            xts[(0, 0)] = x00
            nc.sync.dma_start(out=bqk_sb, in_=bqk[:, :])
            for i in range(4):
                nc.sync.dma_start(out=wq_sb[:, 4 * i:4 * i + 4, :],
                                  in_=wq[:, 1024 * i:1024 * (i + 1)])
                nc.scalar.dma_start(out=x00[:, 4 * i:4 * i + 4, :],
                                    in_=xt[0:128, 1024 * i:1024 * (i + 1)])
            nc.vector.memset(ones_sb, 1.0)
            x_dma(0, 1, eng=nc.scalar)
            for i in range(2):
                nc.sync.dma_start(out=wk_sb[:, 8 * i:8 * i + 8, :],
                                  in_=wk[:, 2048 * i:2048 * (i + 1)])
            for sp in range(2, NSPAN):
                x_dma(0, sp)
            for i in range(2):
                nc.sync.dma_start(out=wv_sb[:, 8 * i:8 * i + 8, :],
                                  in_=wv[:, 2048 * i:2048 * (i + 1)])
            nc.sync.dma_start(out=wo_sb, in_=wo[:, :])

            def pull(filler):
                if filler is not None:
                    next(filler, None)

            def proj_pass(b, w_sb, b_off, dst, spans=None):
                for sp in (spans if spans is not None else range(NSPAN)):
                    xtl = xts[(b, sp)]
                    for h in range(HP):
                        pps = ps.tile([128, SPAN], F32, name="pps", tag="pj",
                                      bufs=2)
                        for c in range(NDC):
                            nc.tensor.matmul(
                                pps, w_sb[:, c, h * HD:(h + 1) * HD],
                                xtl[:, c, :], start=(c == 0),
                                stop=(c == NDC - 1))
                        nc.scalar.activation(
                            dst[:, h, sp * SPAN:(sp + 1) * SPAN], pps, Ident,
                            bias=bqk_sb[:, b_off + h:b_off + h + 1])

            def v_pass(b, v_b):
                for sp in range(NSPAN):
                    xtl = xts[(b, sp)]
                    vps = ps.tile([128, 2 * DC], F32, name="vps", tag="pj",
                                  bufs=2)
                    for tch in range(2):
                        for c in range(NDC):
                            nc.tensor.matmul(
                                vps[:, tch * DC:(tch + 1) * DC],
                                xtl[:, c, tch * 128:(tch + 1) * 128],
                                wv_sb[:, c, :], start=(c == 0),
                                stop=(c == NDC - 1))
                    for tch in range(2):
                        nc.scalar.copy(v_b[:, sp * 2 + tch, :],
                                       vps[:, tch * DC:(tch + 1) * DC])

            def warm_pair(qs, h, kp, qt_b, kt_b):
                # scores+exp of an upcoming half-span pair, emitted before
                # the current tail so the ACT exp stream never restarts cold
                q_sl = qt_b[:, h, qs * QS:(qs + 1) * QS]
                s_ps = ps.tile([128, 2 * QS], F32, name="s_ps", tag="s",
                               bufs=2)
                for j in range(2):
                    kc = 2 * kp + j
                    nc.tensor.matmul(
                        s_ps[:, j * QS:(j + 1) * QS],
                        kt_b[:, h, kc * 128:(kc + 1) * 128], q_sl,
                        start=True, stop=True)
                pt = ppool.tile([128, 2 * QS], F16, name="p_sb", tag="p")
                nc.scalar.activation(pt, s_ps, Exp, scale=SCALE)
                return pt

            def attn_span(qs, h, qt_b, kt_b, v_b, avt_b, filler=None,
                          warm=None, fin=None):
                q_sl = qt_b[:, h, qs * QS:(qs + 1) * QS]
                av_ps = ps.tile([HD, QS], F32, name="av_ps", tag="av",
                                bufs=2)
                p_tiles = []

                def emit_av(kp):
                    pt = p_tiles[kp]
                    for j in range(2):
                        kc = 2 * kp + j
                        nc.tensor.matmul(
                            av_ps, v_b[:, kc, h * HD:(h + 1) * HD],
                            pt[:, j * QS:(j + 1) * QS], start=(kc == 0),
                            stop=(kc == NKC - 1))

                tre = {}

                def s2t(nm, wide=True):
                    return s2pool.tile([128, (2 * QS) if wide else QS], F16,
                                       name=nm, tag="s2")

                for kp in range(NPAIR):
                    if kp == 0 and fin is not None:
                        fin()      # previous span's AV tail + dn + normalize
                    if warm is not None and kp < len(warm):
                        p_tiles.append(warm[kp])
                    else:
                        s_ps = ps.tile([128, 2 * QS], F32, name="s_ps",
                                       tag="s", bufs=2)
                        for j in range(2):
                            kc = 2 * kp + j
                            nc.tensor.matmul(
                                s_ps[:, j * QS:(j + 1) * QS],
                                kt_b[:, h, kc * 128:(kc + 1) * 128], q_sl,
                                start=True, stop=True)
                        pt = ppool.tile([128, 2 * QS], F16, name="p_sb",
                                        tag="p")
                        nc.scalar.activation(pt, s_ps, Exp, scale=SCALE)
                        p_tiles.append(pt)
                    # pair-sum tree: T0/T1 on the idle GPSIMD (latency-
                    # tolerant), combines + half-folds on the DVE, each
                    # emitted at the kp where its inputs are already done so
                    # the ~97%-loaded DVE never dead-waits; x3 (the folded
                    # full denominator row-set) lands ~1.2us after the last
                    # exp and the single ones-matmul runs at the next span's
                    # kp0 slot
                    if kp in (1, 3):
                        tre[kp] = s2t("t_sb")
                        nc.gpsimd.tensor_add(tre[kp], p_tiles[kp - 1],
                                             p_tiles[kp])
                    if kp >= 2:
                        pull(filler)
                    if kp == 4:
                        tre["u"] = s2t("u_sb")
                        nc.vector.tensor_add(tre["u"], tre[1], tre[3])
                    elif kp == 6:
                        tre["t2"] = s2t("t2_sb")
                        nc.vector.tensor_add(tre["t2"], p_tiles[4],
                                             p_tiles[5])
                        tre["x"] = s2t("x_sb")
                        nc.vector.tensor_add(tre["x"], tre["u"], tre["t2"])
                    elif kp == 7:
                        tre["xf"] = s2t("xf_sb", wide=False)
                        nc.vector.tensor_add(tre["xf"], tre["x"][:, 0:QS],
                                             tre["x"][:, QS:2 * QS])
                        tre["a"] = s2t("a_sb", wide=False)
                        nc.vector.tensor_add(tre["a"], p_tiles[6][:, 0:QS],
                                             p_tiles[6][:, QS:2 * QS])
                    if kp == 3:
                        emit_av(0)
                        emit_av(1)
                    elif kp >= 4:
                        emit_av(kp - 2)

                def tail(mid=None):
                    bf = s2t("bf_sb", wide=False)
                    nc.vector.tensor_add(bf, p_tiles[7][:, 0:QS],
                                         p_tiles[7][:, QS:2 * QS])
                    ab = s2t("ab_sb", wide=False)
                    nc.vector.tensor_add(ab, tre["a"], bf)
                    x3 = s2t("x3_sb", wide=False)
                    nc.vector.tensor_add(x3, tre["xf"], ab)
                    if mid is not None:
                        mid()
                    pull(filler)
                    pull(filler)

                    def fin():
                        # deferred span finish, run in the NEXT span's kp0-1
                        # slot where the PE would otherwise starve waiting on
                        # the exp stream: last two AV pairs, then the single
                        # denominator ones-matmul + normalize
                        emit_av(NPAIR - 2)
                        emit_av(NPAIR - 1)
                        dn_ps = ps.tile([128, QS], F32, name="dn_ps",
                                        tag="pj", bufs=2)
                        nc.tensor.matmul(dn_ps, ones_sb, x3, start=True,
                                         stop=True)
                        recip = rpool.tile([128, QS], F32, name="recip",
                                           tag="rc")
                        nc.vector.reciprocal_approx_fast(recip, dn_ps)
                        nc.vector.tensor_mul(
                            avt_b[:, h, qs * QS:(qs + 1) * QS], av_ps, recip)
                    return fin
                return tail

            def outproj_gen(b, qs, avt_b, split, final=False):
                for tloc in range(QS // 128):
                    tch = qs * (QS // 128) + tloc
                    out_sb = opool.tile([128, D], F16, name="out_sb",
                                        tag="ot")
                    rows = slice(b * S + tch * 128, b * S + (tch + 1) * 128)
                    for dsp in range(4):
                        ops = ps.tile([128, 512], F32, name="ops", tag="pj",
                                      bufs=2)
                        for h in range(HP):
                            nc.tensor.matmul(
                                ops, avt_b[:, h, tch * 128:(tch + 1) * 128],
                                wo_sb[:, h, dsp * 512:(dsp + 1) * 512],
                                start=(h == 0), stop=(h == HP - 1))
                        if split[dsp] == "v":
                            nc.vector.tensor_copy(
                                out_sb[:, dsp * 512:(dsp + 1) * 512], ops)
                        elif split[dsp] == "s":
                            nc.scalar.copy(
                                out_sb[:, dsp * 512:(dsp + 1) * 512], ops)
                        else:   # "2": halves on both engines in parallel
                            nc.vector.tensor_copy(
                                out_sb[:, dsp * 512:dsp * 512 + 256],
                                ops[:, 0:256])
                            nc.scalar.copy(
                                out_sb[:, dsp * 512 + 256:(dsp + 1) * 512],
                                ops[:, 256:512])
                        if final:
                            # drain: ship each quarter as soon as it's ready
                            # so the last DMA starts right after the last copy
                            nc.sync.dma_start(
                                out=out[rows, dsp * 512:(dsp + 1) * 512],
                                in_=out_sb[:, dsp * 512:(dsp + 1) * 512])
                        elif dsp == 3:
                            nc.sync.dma_start(out=out[rows, :], in_=out_sb)
                        yield

            carry = None          # half-consumed outproj of (b-1, qs=3)
            prev_fin = None       # previous span's dn+normalize closure
            for b in range(B):
                qt_b = qkpool.tile([128, HP, S], F16, name="qt_b", tag="qt")
                kt_b = qkpool.tile([128, HP, S], F16, name="kt_b", tag="kt")
                v_b = bpool.tile([128, NKC, DC], F16, name="v_b", tag="v")
                avt_b = avpool.tile([128, HP, S], F16, name="avt_b",
                                    tag="avt")

                if b == 0:
                    # first batch is DMA-paced: alternate Q/K half-passes so
                    # the PE never outruns the x-span stream
                    proj_pass(b, wq_sb, 0, qt_b, spans=range(0, 4))
                    proj_pass(b, wk_sb, HP, kt_b, spans=range(0, 4))
                    proj_pass(b, wq_sb, 0, qt_b, spans=range(4, 8))
                    proj_pass(b, wk_sb, HP, kt_b, spans=range(4, 8))
                else:
                    proj_pass(b, wq_sb, 0, qt_b)
                    proj_pass(b, wk_sb, HP, kt_b)
                v_pass(b, v_b)

                warm = [warm_pair(0, 0, 0, qt_b, kt_b),
                        warm_pair(0, 0, 1, qt_b, kt_b)]
                for qs in range(NQS):
                    if qs == 0:
                        filler = carry       # leftovers (may be exhausted)
                    else:
                        filler = outproj_gen(b, qs - 1, avt_b, "vvvv")
                    tail = attn_span(qs, 0, qt_b, kt_b, v_b, avt_b, filler,
                                     warm, fin=prev_fin)
                    warm = [warm_pair(qs, 1, 0, qt_b, kt_b)]
                    prev_fin = tail(mid=lambda: warm.append(
                        warm_pair(qs, 1, 1, qt_b, kt_b)))
                    if qs == 0 and b + 1 < B:
                        for sp in range(NSPAN):
                            x_dma(b + 1, sp)
                    tail = attn_span(qs, 1, qt_b, kt_b, v_b, avt_b, filler,
                                     warm, fin=prev_fin)
                    if qs + 1 < NQS:
                        warm = [warm_pair(qs + 1, 0, 0, qt_b, kt_b)]
                        prev_fin = tail(mid=lambda q=qs: warm.append(
                            warm_pair(q + 1, 0, 1, qt_b, kt_b)))
                    else:
                        warm = None
                        prev_fin = tail()
                    if filler is not None:
                        for _ in filler:     # drain any leftovers
                            pass
                carry = outproj_gen(b, NQS - 1, avt_b,
                                    "vvvv" if b + 1 < B else "svsv",
                                    final=(b + 1 == B))

            if carry is not None:            # last batch's final span:
                prev_fin()                       # normalize its avt first,
                for _ in carry:                  # then drain with copies
                    pass                         # split across ACT+DVE
    nc.compile()
    _BUILT = nc
    return nc


def _install_trace_hooks():
    import types
    try:
        import antenv.axon_hooks  # noqa: F401
        return True
    except ImportError:
        pass
    try:
        from trn_agent_boot.trn_boot import _ntff_profile_via_ctypes
        hook = _ntff_profile_via_ctypes('/opt/axon/libaxon_pjrt.so')
        if hook is None:
            return False
        m = types.ModuleType('antenv.axon_hooks')
        m.get_axon_ntff_profile_hook = lambda: hook
        sys.modules['antenv.axon_hooks'] = m
        from concourse import bass_utils
        bass_utils.upload_artifacts = lambda tmpdir: "local://" + tmpdir
        return True
    except Exception:
        return False


def kernel(x, wq, bq, wk, bk, wv, bv, wo, bo):
    global LAST_EXEC_NS
    from concourse.bass_utils import run_bass_kernel_spmd

    x = np.asarray(x, dtype=np.float32)
    wq = np.asarray(wq, dtype=np.float32)
    bq = np.asarray(bq, dtype=np.float32)
    wk = np.asarray(wk, dtype=np.float32)
    bk = np.asarray(bk, dtype=np.float32)
    wv = np.asarray(wv, dtype=np.float32)
    bv = np.asarray(bv, dtype=np.float32)
    wo = np.asarray(wo, dtype=np.float32)
    bo = np.asarray(bo, dtype=np.float32)

    # xt blocks: [(b,sp), 128 part, NDC, SPAN] with contiguous 8KB
    # per-partition rows; element (b, sp, p, c, t) = x[b, sp*SPAN+t, c*128+p]
    xt = np.ascontiguousarray(
        x.reshape(B, NSPAN, SPAN, NDC, 128).transpose(0, 1, 4, 3, 2)
    ).astype(np.float16).reshape(B * NSPAN * 128, NDC * SPAN)

    def w_prep(w):  # [D, DC] -> [128, NDC*DC] (partition-major, contiguous)
        return np.ascontiguousarray(
            w.reshape(NDC, 128, DC).transpose(1, 0, 2)
        ).astype(np.float16).reshape(128, NDC * DC)

    in_maps = []
    for i in range(NCORES):
        sl = slice(i * DC, (i + 1) * DC)
        in_maps.append({
            "xt": xt,
            "wq": w_prep(wq[:, sl]),
            "wk": w_prep(wk[:, sl]),
            "wv": w_prep(wv[:, sl]),
            "wo": np.ascontiguousarray(
                wo[sl, :].reshape(HP, 128, D).transpose(1, 0, 2)
            ).astype(np.float16).reshape(128, HP * D),
            "bqk": np.ascontiguousarray(np.concatenate(
                [bq[sl].reshape(HP, HD).T, bk[sl].reshape(HP, HD).T],
                axis=1)),
        })

    trace = bool(os.environ.get("KERNEL_TRACE"))
    if trace:
        trace = _install_trace_hooks()

    nc = _build()
    res = run_bass_kernel_spmd(nc, in_maps, list(range(NCORES)), trace=trace)
    LAST_EXEC_NS = res.exec_time_ns

    total = np.zeros((TOK, D), dtype=np.float32)
    for r in res.results:
        total += r["out"]
    # V-bias folds into a constant row: softmax rows sum to 1, so
    # attention(V + 1*bv^T) = attention(V) + 1*bv^T, and (bv @ wo) adds to bo.
    total += bo + bv @ wo
    return total.reshape(B, S, D)



# revision 28
# speedup vs baseline: 1.1260x; 1.0463x over previous
"""Multi-head attention (B=4, S=2048, D=2048, H=16) on 8 trn2 NeuronCores.

Sharding: tensor-parallel over heads - 2 heads per core. Each core computes
its heads' Q/K/V projections, full attention for those heads, and a partial
output projection (its 256 rows of wo). The host sums the 8 fp16 partial
outputs (plus the bias terms: bv folds through softmax-rows-sum-to-1, bo is
added once).

Design (947us baseline -> 767us):
  - fp16 datapath: every matmul operand (x, weights, Q/K/V, exp tiles,
    attention output) is fp16; PSUM accumulation stays fp32. Same relative
    precision as tf32 (10 mantissa bits) at half the DMA/SBUF/LDWEIGHTS
    cost. rel_err ~1.2e-3 vs the 2e-2 gate.
  - softmax denominator off the PE: exp tiles are pair-summed on the
    otherwise idle GPSIMD (T0=p0+p1, T1=p2+p3; latency-tolerant), combined
    on the DVE (U=T0+T1, T2=p4+p5, X=U+T2, then T3=p6+p7, X2=X+T3 and a
    half-fold X3 in the tail), so a SINGLE ones-matmul per half-span — run
    in the next span's kp0 slot, where the avt consumer is still ~10us
    away — partition-reduces the whole denominator. This replaces the
    original 512 denominator matmuls (~119us of PE; the previous revision
    still used 6 per half-span, ~41us).
  - the phase-B pair loop is paced by the ACT exp stream (1113ns per
    [128,1024] exp + ~180ns dispatch restart). The PE is kept saturated
    against that cadence: the out-projection of span qs-1 is emitted as 16
    two-matmul groups pulled into span qs's pair loops, and each upcoming
    half-span's first two scores+exp pairs are emitted before the current
    tail ("warm pairs") so the exp stream never restarts cold across
    spans, heads, or batches.
  - PSUM (8 banks): scores ring 2x[128,1024] (4), av ring 2 (2, its own
    tag so a new half-span's AV never waits the previous normalize),
    proj/outproj/dn shared ring 2 (2).
  - PSUM->SBUF output copies ride the DVE; the final drain splits each
    copy across DVE+ACT halves (both idle at the end).
  - b=0 startup: wq chunk-DMAs interleave with the first x span; Q/K run
    as alternating half-passes so the PE never outruns the 320GB/s x
    stream; first matmul at ~12.5us.
"""
import os
import sys

sys.path.insert(0, "/opt/trn_rl_repo")
import numpy as np

B, S, D, H = 4, 2048, 2048, 16
HD = 128
NCORES = 8
HP = H // NCORES          # heads per core = 2
DC = HP * HD              # per-core slice of D = 256
TOK = B * S               # 8192
SCALE = HD ** -0.5
NDC = D // 128            # 16 contraction chunks for the projections
SPAN = 256                # token span per projection step
NSPAN = S // SPAN         # 8 spans per batch
QS = 512                  # query span in attention
NQS = S // QS             # 4
NKC = S // 128            # 16 key chunks
NPAIR = NKC // 2          # 8 key-chunk pairs

LAST_EXEC_NS = None
_BUILT = None


def _build():
    global _BUILT
    if _BUILT is not None:
        return _BUILT
    import concourse.tile as tile
    from concourse import bacc, mybir

    F16 = mybir.dt.float16
    F32 = mybir.dt.float32
    Exp = mybir.ActivationFunctionType.Exp
    Ident = mybir.ActivationFunctionType.Identity

    nc = bacc.Bacc("TRN2", target_bir_lowering=False, debug=False)
    # All inputs are pre-swizzled on the host so every DMA reads contiguous
    # multi-KB per-partition rows (512B strided pieces run at ~40% of peak).
    # xt: per (batch,span) block of [128 part, NDC*SPAN] contiguous.
    xt = nc.dram_tensor("xt", [B * NSPAN * 128, NDC * SPAN], F16,
                        kind="ExternalInput")
    wq = nc.dram_tensor("wq", [128, NDC * DC], F16, kind="ExternalInput")
    wk = nc.dram_tensor("wk", [128, NDC * DC], F16, kind="ExternalInput")
    wv = nc.dram_tensor("wv", [128, NDC * DC], F16, kind="ExternalInput")
    wo = nc.dram_tensor("wo", [128, HP * D], F16, kind="ExternalInput")
    bqk = nc.dram_tensor("bqk", [HD, 2 * HP], F32, kind="ExternalInput")
    out = nc.dram_tensor("out", [TOK, D], F16, kind="ExternalOutput")

    with tile.TileContext(nc) as tc:
        with tc.tile_pool(name="const", bufs=1) as cpool, \
             tc.tile_pool(name="xp", bufs=8) as xpool, \
             tc.tile_pool(name="bt", bufs=2) as bpool, \
             tc.tile_pool(name="qk", bufs=2) as qkpool, \
             tc.tile_pool(name="avp", bufs=2) as avpool, \
             tc.tile_pool(name="pp", bufs=5) as ppool, \
             tc.tile_pool(name="s2", bufs=8) as s2pool, \
             tc.tile_pool(name="rc", bufs=2) as rpool, \
             tc.tile_pool(name="ot", bufs=2) as opool, \
             tc.tile_pool(name="ps", bufs=1, space="PSUM") as ps:

            wq_sb = cpool.tile([128, NDC, DC], F16)
            wk_sb = cpool.tile([128, NDC, DC], F16)
            wv_sb = cpool.tile([128, NDC, DC], F16)
            wo_sb = cpool.tile([128, HP, D], F16)
            ones_sb = cpool.tile([128, 128], F16)
            bqk_sb = cpool.tile([HD, 2 * HP], F32)

            xts = {}

            def x_dma(b, sp, eng=None):
                r0 = (b * NSPAN + sp) * 128
                xtl = xpool.tile([128, NDC, SPAN], F16, name=f"x{b}_{sp}",
                                 tag="x")
                xts[(b, sp)] = xtl
                (eng or nc.sync).dma_start(out=xtl, in_=xt[r0:r0 + 128, :])

            # --- b=0 startup. A HWDGE dma_start occupies the issuing
            # engine's queue and can BLOCK it waiting for a free DMA sem
            # lane, so the scalar (ACT) queue gets only the first x spans —
            # ACT must be free for the first proj activations by ~12us.
            # Everything else rides sync in need-order: bias (first act),
            # wq, wk (K pass ~25us), x2-x7 (~22-45us), wv (~65), wo (~100).
            x00 = xpool.tile([128, NDC, SPAN], F16, name="x0_0", tag="x")
            xts[(0, 0)] = x00
            nc.sync.dma_start(out=bqk_sb, in_=bqk[:, :])
            for i in range(4):
                nc.sync.dma_start(out=wq_sb[:, 4 * i:4 * i + 4, :],
                                  in_=wq[:, 1024 * i:1024 * (i + 1)])
                nc.scalar.dma_start(out=x00[:, 4 * i:4 * i + 4, :],
                                    in_=xt[0:128, 1024 * i:1024 * (i + 1)])
            nc.vector.memset(ones_sb, 1.0)
            x_dma(0, 1, eng=nc.scalar)
            for i in range(2):
                nc.sync.dma_start(out=wk_sb[:, 8 * i:8 * i + 8, :],
                                  in_=wk[:, 2048 * i:2048 * (i + 1)])
            for sp in range(2, NSPAN):
                x_dma(0, sp)
            for i in range(2):
                nc.sync.dma_start(out=wv_sb[:, 8 * i:8 * i + 8, :],
                                  in_=wv[:, 2048 * i:2048 * (i + 1)])
            nc.sync.dma_start(out=wo_sb, in_=wo[:, :])

            def pull(filler):
                if filler is not None:
                    next(filler, None)

            def proj_pass(b, w_sb, b_off, dst, spans=None):
                for sp in (spans if spans is not None else range(NSPAN)):
                    xtl = xts[(b, sp)]
                    for h in range(HP):
                        pps = ps.tile([128, SPAN], F32, name="pps", tag="pj",
                                      bufs=2)
                        for c in range(NDC):
                            nc.tensor.matmul(
                                pps, w_sb[:, c, h * HD:(h + 1) * HD],
                                xtl[:, c, :], start=(c == 0),
                                stop=(c == NDC - 1))
                        nc.scalar.activation(
                            dst[:, h, sp * SPAN:(sp + 1) * SPAN], pps, Ident,
                            bias=bqk_sb[:, b_off + h:b_off + h + 1])

            def v_pass(b, v_b):
                for sp in range(NSPAN):
                    xtl = xts[(b, sp)]
                    vps = ps.tile([128, 2 * DC], F32, name="vps", tag="pj",
                                  bufs=2)
                    for tch in range(2):
                        for c in range(NDC):
                            nc.tensor.matmul(
                                vps[:, tch * DC:(tch + 1) * DC],
                                xtl[:, c, tch * 128:(tch + 1) * 128],
                                wv_sb[:, c, :], start=(c == 0),
                                stop=(c == NDC - 1))
                    for tch in range(2):
                        nc.scalar.copy(v_b[:, sp * 2 + tch, :],
                                       vps[:, tch * DC:(tch + 1) * DC])

            def warm_pair(qs, h, kp, qt_b, kt_b):
                # scores+exp of an upcoming half-span pair, emitted before
                # the current tail so the ACT exp stream never restarts cold
                q_sl = qt_b[:, h, qs * QS:(qs + 1) * QS]
                s_ps = ps.tile([128, 2 * QS], F32, name="s_ps", tag="s",
                               bufs=2)
                for j in range(2):
                    kc = 2 * kp + j
                    nc.tensor.matmul(
                        s_ps[:, j * QS:(j + 1) * QS],
                        kt_b[:, h, kc * 128:(kc + 1) * 128], q_sl,
                        start=True, stop=True)
                pt = ppool.tile([128, 2 * QS], F16, name="p_sb", tag="p")
                nc.scalar.activation(pt, s_ps, Exp, scale=SCALE)
                return pt

            def attn_span(qs, h, qt_b, kt_b, v_b, avt_b, filler=None,
                          warm=None, fin=None):
                q_sl = qt_b[:, h, qs * QS:(qs + 1) * QS]
                av_ps = ps.tile([HD, QS], F32, name="av_ps", tag="av",
                                bufs=2)
                p_tiles = []

                def emit_av(kp):
                    pt = p_tiles[kp]
                    for j in range(2):
                        kc = 2 * kp + j
                        nc.tensor.matmul(
                            av_ps, v_b[:, kc, h * HD:(h + 1) * HD],
                            pt[:, j * QS:(j + 1) * QS], start=(kc == 0),
                            stop=(kc == NKC - 1))

                tre = {}

                def s2t(nm, wide=True):
                    return s2pool.tile([128, (2 * QS) if wide else QS], F16,
                                       name=nm, tag="s2")

                for kp in range(NPAIR):
                    if kp == 0 and fin is not None:
                        fin()      # previous span's AV tail + dn + normalize
                    if warm is not None and kp < len(warm):
                        p_tiles.append(warm[kp])
                    else:
                        s_ps = ps.tile([128, 2 * QS], F32, name="s_ps",
                                       tag="s", bufs=2)
                        for j in range(2):
                            kc = 2 * kp + j
                            nc.tensor.matmul(
                                s_ps[:, j * QS:(j + 1) * QS],
                                kt_b[:, h, kc * 128:(kc + 1) * 128], q_sl,
                                start=True, stop=True)
                        pt = ppool.tile([128, 2 * QS], F16, name="p_sb",
                                        tag="p")
                        nc.scalar.activation(pt, s_ps, Exp, scale=SCALE)
                        p_tiles.append(pt)
                    # pair-sum tree: T0/T1 on the idle GPSIMD (latency-
                    # tolerant), wide combines on the DVE, each emitted at
                    # the kp where its inputs are already done so the
                    # heavily-loaded DVE never dead-waits. Dependent chains
                    # of DVE ops pay ~1us+ per hop (drain + sem latency), so
                    # after the last exp there is exactly ONE hop (X4=X5+p7)
                    # before the two denominator ones-matmuls in the next
                    # span's kp0 slot.
                    if kp in (1, 3):
                        tre[kp] = s2t("t_sb")
                        nc.gpsimd.tensor_add(tre[kp], p_tiles[kp - 1],
                                             p_tiles[kp])
                    if kp >= 2:
                        pull(filler)
                    if kp == 5:
                        tre["u"] = s2t("u_sb")
                        nc.vector.tensor_add(tre["u"], tre[1], tre[3])
                    elif kp == 6:
                        tre["t2"] = s2t("t2_sb")
                        nc.vector.tensor_add(tre["t2"], p_tiles[4],
                                             p_tiles[5])
                        tre["x"] = s2t("x_sb")
                        nc.vector.tensor_add(tre["x"], tre["u"], tre["t2"])
                    elif kp == 7:
                        tre["x5"] = s2t("x5_sb")
                        nc.vector.tensor_add(tre["x5"], tre["x"],
                                             p_tiles[6])
                    if kp == 3:
                        emit_av(0)
                        emit_av(1)
                    elif kp >= 4:
                        emit_av(kp - 2)

                def tail(mid=None):
                    x4 = s2t("x4_sb")
                    nc.vector.tensor_add(x4, tre["x5"], p_tiles[7])
                    if mid is not None:
                        mid()
                    pull(filler)
                    pull(filler)

                    def fin():
                        # deferred span finish, run in the NEXT span's kp0-1
                        # slot where the PE would otherwise starve waiting on
                        # the exp stream: last two AV pairs, then the two
                        # denominator ones-matmuls + normalize
                        emit_av(NPAIR - 2)
                        emit_av(NPAIR - 1)
                        dn_ps = ps.tile([128, QS], F32, name="dn_ps",
                                        tag="pj", bufs=2)
                        nc.tensor.matmul(dn_ps, ones_sb, x4[:, 0:QS],
                                         start=True, stop=False)
                        nc.tensor.matmul(dn_ps, ones_sb, x4[:, QS:2 * QS],
                                         start=False, stop=True)
                        recip = rpool.tile([128, QS], F32, name="recip",
                                           tag="rc")
                        nc.vector.reciprocal_approx_fast(recip, dn_ps)
                        nc.vector.tensor_mul(
                            avt_b[:, h, qs * QS:(qs + 1) * QS], av_ps, recip)
                    return fin
                return tail

            def outproj_gen(b, qs, avt_b, split, final=False):
                for tloc in range(QS // 128):
                    tch = qs * (QS // 128) + tloc
                    out_sb = opool.tile([128, D], F16, name="out_sb",
                                        tag="ot")
                    rows = slice(b * S + tch * 128, b * S + (tch + 1) * 128)
                    for dsp in range(4):
                        ops = ps.tile([128, 512], F32, name="ops", tag="pj",
                                      bufs=2)
                        for h in range(HP):
                            nc.tensor.matmul(
                                ops, avt_b[:, h, tch * 128:(tch + 1) * 128],
                                wo_sb[:, h, dsp * 512:(dsp + 1) * 512],
                                start=(h == 0), stop=(h == HP - 1))
                        if split[dsp] == "v":
                            nc.vector.tensor_copy(
                                out_sb[:, dsp * 512:(dsp + 1) * 512], ops)
                        elif split[dsp] == "s":
                            nc.scalar.copy(
                                out_sb[:, dsp * 512:(dsp + 1) * 512], ops)
                        else:   # "2": halves on both engines in parallel
                            nc.vector.tensor_copy(
                                out_sb[:, dsp * 512:dsp * 512 + 256],
                                ops[:, 0:256])
                            nc.scalar.copy(
                                out_sb[:, dsp * 512 + 256:(dsp + 1) * 512],
                                ops[:, 256:512])
                        if final:
                            # drain: ship each quarter as soon as it's ready
                            # so the last DMA starts right after the last copy
                            nc.sync.dma_start(
                                out=out[rows, dsp * 512:(dsp + 1) * 512],
                                in_=out_sb[:, dsp * 512:(dsp + 1) * 512])
                        elif dsp == 3:
                            nc.sync.dma_start(out=out[rows, :], in_=out_sb)
                        yield

            carry = None          # half-consumed outproj of (b-1, qs=3)
            prev_fin = None       # previous span's dn+normalize closure
            for b in range(B):
                qt_b = qkpool.tile([128, HP, S], F16, name="qt_b", tag="qt")
                kt_b = qkpool.tile([128, HP, S], F16, name="kt_b", tag="kt")
                v_b = bpool.tile([128, NKC, DC], F16, name="v_b", tag="v")
                avt_b = avpool.tile([128, HP, S], F16, name="avt_b",
                                    tag="avt")

                if b == 0:
                    # first batch is DMA-paced: alternate Q/K half-passes so
                    # the PE never outruns the x-span stream
                    proj_pass(b, wq_sb, 0, qt_b, spans=range(0, 4))
                    proj_pass(b, wk_sb, HP, kt_b, spans=range(0, 4))
                    proj_pass(b, wq_sb, 0, qt_b, spans=range(4, 8))
                    proj_pass(b, wk_sb, HP, kt_b, spans=range(4, 8))
                else:
                    proj_pass(b, wq_sb, 0, qt_b)
                    proj_pass(b, wk_sb, HP, kt_b)
                v_pass(b, v_b)

                warm = [warm_pair(0, 0, 0, qt_b, kt_b),
                        warm_pair(0, 0, 1, qt_b, kt_b)]
                for qs in range(NQS):
                    if qs == 0:
                        filler = carry       # leftovers (may be exhausted)
                    else:
                        filler = outproj_gen(b, qs - 1, avt_b, "vvvv")
                    tail = attn_span(qs, 0, qt_b, kt_b, v_b, avt_b, filler,
                                     warm, fin=prev_fin)
                    warm = [warm_pair(qs, 1, 0, qt_b, kt_b)]
                    prev_fin = tail(mid=lambda: warm.append(
                        warm_pair(qs, 1, 1, qt_b, kt_b)))
                    if qs == 0 and b + 1 < B:
                        for sp in range(NSPAN):
                            x_dma(b + 1, sp)
                    tail = attn_span(qs, 1, qt_b, kt_b, v_b, avt_b, filler,
                                     warm, fin=prev_fin)
                    if qs + 1 < NQS:
                        warm = [warm_pair(qs + 1, 0, 0, qt_b, kt_b)]
                        prev_fin = tail(mid=lambda q=qs: warm.append(
                            warm_pair(q + 1, 0, 1, qt_b, kt_b)))
                    else:
                        warm = None
                        prev_fin = tail()
                    if filler is not None:
                        for _ in filler:     # drain any leftovers
                            pass
                carry = outproj_gen(b, NQS - 1, avt_b,
                                    "vvvv" if b + 1 < B else "svsv",
                                    final=(b + 1 == B))

            if carry is not None:            # last batch's final span:
                prev_fin()                       # normalize its avt first,
                for _ in carry:                  # then drain with copies
                    pass                         # split across ACT+DVE
    nc.compile()
    _BUILT = nc
    return nc


def _install_trace_hooks():
    import types
    try:
        import antenv.axon_hooks  # noqa: F401
        return True
    except ImportError:
        pass
    try:
        from trn_agent_boot.trn_boot import _ntff_profile_via_ctypes
        hook = _ntff_profile_via_ctypes('/opt/axon/libaxon_pjrt.so')
        if hook is None:
            return False
        m = types.ModuleType('antenv.axon_hooks')
        m.get_axon_ntff_profile_hook = lambda: hook
        sys.modules['antenv.axon_hooks'] = m
        from concourse import bass_utils
        bass_utils.upload_artifacts = lambda tmpdir: "local://" + tmpdir
        return True
    except Exception:
        return False


def kernel(x, wq, bq, wk, bk, wv, bv, wo, bo):
    global LAST_EXEC_NS
    from concourse.bass_utils import run_bass_kernel_spmd

    x = np.asarray(x, dtype=np.float32)
    wq = np.asarray(wq, dtype=np.float32)
    bq = np.asarray(bq, dtype=np.float32)
    wk = np.asarray(wk, dtype=np.float32)
    bk = np.asarray(bk, dtype=np.float32)
    wv = np.asarray(wv, dtype=np.float32)
    bv = np.asarray(bv, dtype=np.float32)
    wo = np.asarray(wo, dtype=np.float32)
    bo = np.asarray(bo, dtype=np.float32)

    # xt blocks: [(b,sp), 128 part, NDC, SPAN] with contiguous 8KB
    # per-partition rows; element (b, sp, p, c, t) = x[b, sp*SPAN+t, c*128+p]
    xt = np.ascontiguousarray(
        x.reshape(B, NSPAN, SPAN, NDC, 128).transpose(0, 1, 4, 3, 2)
    ).astype(np.float16).reshape(B * NSPAN * 128, NDC * SPAN)

    def w_prep(w):  # [D, DC] -> [128, NDC*DC] (partition-major, contiguous)
        return np.ascontiguousarray(
            w.reshape(NDC, 128, DC).transpose(1, 0, 2)
        ).astype(np.float16).reshape(128, NDC * DC)

    in_maps = []
    for i in range(NCORES):
        sl = slice(i * DC, (i + 1) * DC)
        in_maps.append({
            "xt": xt,
            "wq": w_prep(wq[:, sl]),
            "wk": w_prep(wk[:, sl]),
            "wv": w_prep(wv[:, sl]),
            "wo": np.ascontiguousarray(
                wo[sl, :].reshape(HP, 128, D).transpose(1, 0, 2)
            ).astype(np.float16).reshape(128, HP * D),
            "bqk": np.ascontiguousarray(np.concatenate(
                [bq[sl].reshape(HP, HD).T, bk[sl].reshape(HP, HD).T],
                axis=1)),
        })

    trace = bool(os.environ.get("KERNEL_TRACE"))
    if trace:
        trace = _install_trace_hooks()

    nc = _build()
    res = run_bass_kernel_spmd(nc, in_maps, list(range(NCORES)), trace=trace)
    LAST_EXEC_NS = res.exec_time_ns

    total = np.zeros((TOK, D), dtype=np.float32)
    for r in res.results:
        total += r["out"]
    # V-bias folds into a constant row: softmax rows sum to 1, so
    # attention(V + 1*bv^T) = attention(V) + 1*bv^T, and (bv @ wo) adds to bo.
    total += bo + bv @ wo
    return total.reshape(B, S, D)

